# revision 1
# baseline (speedup 1.0000x reference)
"""BinarizeLinear kernel for TRN2: out = x @ sign(W).

x: [32768, 512] f32, W: [512, 512] f32 -> out: [32768, 512] f32.

Data-parallel across 8 NeuronCores: each core handles 4096 tokens, W is
replicated. Per core:
  - x macro tiles (512 tokens = 1 MiB) are loaded with SWDGE
    cast-during-DMA (gpsimd), arriving in SBUF as fp16 (2^-11 rounding;
    fp32 matmuls would run at 1/4 PE rate, and sign(W) in {-1,0,1} is
    exact in fp16).
  - TensorE transposes each [128 tok, 128 din] block (PE contracts over
    the partition dim, so x must present d_in on partitions; fp16
    transposes run 1 cyc/row vs 2 for fp32).
  - DVE drains the transposed tiles PSUM->SBUF; fp16 matmuls accumulate
    [128 tok, 512 dout] fp32 tiles in PSUM (4 k-chunk accumulation).
  - ScalarE (otherwise idle) casts out tiles PSUM->SBUF fp16; stores go
    out as 0.5 MiB DMAs; host concatenates shards and casts to fp32.
"""

import sys

if "/opt/trn_rl_repo" not in sys.path:
    sys.path.insert(0, "/opt/trn_rl_repo")

import json

import numpy as np

import concourse.bass as bass
import concourse.mybir as mybir
import concourse.tile as tile
from concourse.bass import ds
from concourse.masks import make_identity

# ---------------------------------------------------------------------------
# Workaround: the pinned walrus only accepts ONE sync wait and ONE sync
# update per instruction ("Too many sync wait commands" in setupSyncWait),
# but Tile's kernel-tail Drain carries one wait per outstanding semaphore.
# Split extras onto single-wait NoOps before (waits) / after (updates) the
# instruction — same engine, so program order preserves the semantics.
# ---------------------------------------------------------------------------

_split_uid = 0


def _split_sync(bir_json: bytes) -> bytes:
    global _split_uid
    bir = json.loads(bir_json)
    changed = False
    for fn in bir.get("functions", []):
        for blk in fn.get("blocks", []):
            insts = blk.get("instructions", [])
            out = []
            for inst in insts:
                si = inst.get("sync_info") or {}
                waits = si.get("on_wait") or []
                updates = si.get("on_update") or []
                if len(waits) > 1:
                    for w in waits[:-1]:
                        _split_uid += 1
                        out.append(
                            {
                                "name": f"I-syncsplit-w{_split_uid}",
                                "engine": inst["engine"],
                                "opcode": "NoOp",
                                "ins": [],
                                "outs": [],
                                "sync_info": {"on_update": [], "on_wait": [w]},
                            }
                        )
                    si["on_wait"] = [waits[-1]]
                    changed = True
                out.append(inst)
                if len(updates) > 1:
                    si["on_update"] = [updates[0]]
                    for u in updates[1:]:
                        _split_uid += 1
                        out.append(
                            {
                                "name": f"I-syncsplit-u{_split_uid}",
                                "engine": inst["engine"],
                                "opcode": "NoOp",
                                "ins": [],
                                "outs": [],
                                "sync_info": {"on_update": [u], "on_wait": []},
                            }
                        )
                    changed = True
            blk["instructions"] = out
    if not changed:
        return bir_json
    return json.dumps(bir).encode()


def _install_sync_split_patch() -> None:
    import concourse.bass2jax as bass2jax
    import concourse.bass_utils as bass_utils

    orig = bass_utils.compile_bir_kernel
    if getattr(orig, "_sync_split_patched", False):
        return

    def patched(bir_json, tmpdir, neff_name="file.neff", **kw):
        return orig(_split_sync(bir_json), tmpdir, neff_name, **kw)

    patched._sync_split_patched = True
    bass_utils.compile_bir_kernel = patched
    bass2jax.compile_bir_kernel = patched


_install_sync_split_patch()

N_CORES = 8
N_TOKENS = 32768
D_IN = 512
D_OUT = 512

TOK_PER_CORE = N_TOKENS // N_CORES  # 4096
P = 128  # partitions
K_CHUNKS = D_IN // P  # 4
import os as _os

MACRO = int(_os.environ.get("K_MACRO", "4"))  # token tiles per DMA batch (4 -> 1 MiB)

F32 = mybir.dt.float32
# fp16 for the matmul operands and the output store: sign(W) is exact in
# fp16, x ~ N(0,1) casts with 2^-11 rel error (8x better than bf16), and
# out (|.|<~150 << 65504) stores in half the bytes of f32.
F16 = mybir.dt.float16


def build_kernel(nc: bass.Bass, repeat: int = 1, macro: int | None = None) -> None:
    MACRO = macro if macro is not None else globals()["MACRO"]
    N_MACRO = TOK_PER_CORE // (MACRO * P)
    x = nc.dram_tensor("x", [TOK_PER_CORE, D_IN], F32, kind="ExternalInput").ap()
    w = nc.dram_tensor("W", [D_IN, D_OUT], F32, kind="ExternalInput").ap()
    out = nc.dram_tensor("out", [TOK_PER_CORE, D_OUT], F16, kind="ExternalOutput").ap()

    # [p, a, d] view: token t = a*128 + p within a macro block of 512 tokens
    x_v = x.rearrange("(a p) d -> p a d", p=P)  # [128, 32, 512]
    out_v = out.rearrange("(a p) d -> p a d", p=P)  # [128, 32, 512]
    w_v = w.rearrange("(k p) d -> p k d", p=P)  # [128, 4, 512]

    with tile.TileContext(nc) as tc:
        with (
            tc.tile_pool(name="const", bufs=1) as const_pool,
            tc.tile_pool(name="xin", bufs=6) as xin_pool,
            tc.tile_pool(name="xt", bufs=4) as xt_pool,
            tc.tile_pool(name="outsb", bufs=3) as out_pool,
            tc.tile_pool(name="xt_ps", bufs=4, space="PSUM") as xtps_pool,
            tc.tile_pool(name="out_ps", bufs=4, space="PSUM") as outps_pool,
        ):
            # --- constants: identity for PE transpose, binarized weight ---
            ident = const_pool.tile([P, P], F16)
            make_identity(nc, ident[:])

            # Boot path: the very first token tile is loaded f32 via HWDGE
            # (sync), which has ~0.4us lower first-byte latency than the
            # SWDGE cast path, and is transposed in f32 directly. Everything
            # else streams through SWDGE cast-DMA. The first macro is also
            # split into per-tile DMAs so the first transpose starts after
            # 256 KiB instead of 1 MiB.
            ident32 = const_pool.tile([P, P], F32)
            make_identity(nc, ident32[:])
            first_x32 = const_pool.tile([P, D_IN], F32)
            nc.sync.dma_start(first_x32[:], x_v[:, 0, :])

            first_xin = xin_pool.tile([P, MACRO, D_IN], F16, tag="xin")
            for a in range(1, MACRO):
                nc.gpsimd.dma_start(first_xin[:, a, :], x_v[:, a, :])

            w_f32 = const_pool.tile([P, K_CHUNKS, D_OUT], F32)
            nc.sync.dma_start(w_f32[:], w_v[:])
            w_b = const_pool.tile([P, K_CHUNKS, D_OUT], F16)
            for k in range(K_CHUNKS):
                # sign(w): ACT LUT; +-1/0 are exact in fp16
                nc.scalar.activation(
                    w_b[:, k, :], w_f32[:, k, :], mybir.ActivationFunctionType.Sign
                )

            # --- main loop: 8 macro blocks of 512 tokens ---
            for i, j in enumerate(
                [jj for _ in range(repeat) for jj in range(N_MACRO)]
            ):
                if i == 0:
                    xin = first_xin
                else:
                    xin = xin_pool.tile([P, MACRO, D_IN], F16, tag="xin")
                    nc.gpsimd.dma_start(xin[:], x_v[:, ds(j * MACRO, MACRO), :])

                out_sb = out_pool.tile([P, MACRO, D_OUT], F16)

                for a in range(MACRO):
                    # x arrives f16 (SWDGE cast-during-DMA); transpose
                    # [128 tok, 512 din] -> 4x [128 din, 128 tok]
                    boot = i == 0 and a == 0
                    src_t = first_x32 if boot else xin[:, a, :]
                    xt_ps = xtps_pool.tile([P, D_IN], F32 if boot else F16,
                                           tag="xt_ps")
                    for k in range(K_CHUNKS):
                        nc.tensor.transpose(
                            xt_ps[:, ds(k * P, P)],
                            src_t[:, ds(k * P, P)] if boot
                            else xin[:, a, ds(k * P, P)],
                            ident32[:] if boot else ident[:],
                        )
                    xt_sb = xt_pool.tile([P, D_IN], F16)
                    nc.vector.tensor_copy(xt_sb[:], xt_ps[:])

                    out_ps = outps_pool.tile([P, D_OUT], F32)
                    for k in range(K_CHUNKS):
                        nc.tensor.matmul(
                            out_ps[:],
                            xt_sb[:, ds(k * P, P)],
                            w_b[:, k, :],
                            start=(k == 0),
                            stop=(k == K_CHUNKS - 1),
                        )
                    # out copy on ACT (otherwise idle), freeing DVE for the
                    # cast + xT copies
                    nc.scalar.activation(
                        out_sb[:, a, :],
                        out_ps[:],
                        mybir.ActivationFunctionType.Copy,
                    )

                if i == repeat * N_MACRO - 1:
                    # per-tile stores at the end: each store departs as soon
                    # as its ACT copy lands, shortening the pipeline flush
                    for a in range(MACRO):
                        nc.sync.dma_start(
                            out_v[:, j * MACRO + a, :], out_sb[:, a, :]
                        )
                else:
                    nc.sync.dma_start(
                        out_v[:, ds(j * MACRO, MACRO), :], out_sb[:]
                    )


def _build_nc(repeat: int = 1, macro: int | None = None) -> bass.Bass:
    nc = bass.Bass(
        "TRN2",
        target_bir_lowering=False,
        debug=False,
        num_devices=N_CORES,
    )
    build_kernel(nc, repeat=repeat, macro=macro)
    return nc


_NC_CACHE = None
_FN_CACHE = None


def _get_callable():
    """Build (once) a jitted shard_map callable over the 8 cores.

    Mirrors bass2jax.run_bass_via_pjrt's multi-core path, but cached so
    repeated kernel() calls reuse the compiled executable instead of
    re-tracing a fresh closure every time.
    """
    global _NC_CACHE, _FN_CACHE
    if _FN_CACHE is not None:
        return _FN_CACHE

    import jax
    from jax.experimental.shard_map import shard_map
    from jax.sharding import Mesh, PartitionSpec

    from concourse import bass2jax

    bass2jax.install_neuronx_cc_hook()

    if _NC_CACHE is None:
        _NC_CACHE = _build_nc()
    nc = _NC_CACHE

    partition_name = nc.partition_id_tensor.name if nc.partition_id_tensor else None
    in_names, out_names, out_avals, zero_outs = [], [], [], []
    for alloc in nc.m.functions[0].allocations:
        if not isinstance(alloc, mybir.MemoryLocationSet):
            continue
        name = alloc.memorylocations[0].name
        if alloc.kind == "ExternalInput":
            if name != partition_name:
                in_names.append(name)
        elif alloc.kind == "ExternalOutput":
            shape = tuple(alloc.tensor_shape)
            dtype = mybir.dt.np(alloc.dtype)
            out_names.append(name)
            out_avals.append(jax.core.ShapedArray(shape, dtype))
            zero_outs.append(np.zeros(shape, dtype))
    all_in_names = in_names + out_names
    if partition_name is not None:
        all_in_names = all_in_names + [partition_name]

    def _body(*args):
        operands = list(args)
        if partition_name is not None:
            operands.append(bass2jax.partition_id_tensor())
        return tuple(
            bass2jax._bass_exec_p.bind(
                *operands,
                out_avals=tuple(out_avals),
                in_names=tuple(all_in_names),
                out_names=tuple(out_names),
                lowering_input_output_aliases=(),
                sim_require_finite=True,
                sim_require_nnan=True,
                nc=nc,
            )
        )

    devices = jax.devices()[:N_CORES]
    mesh = Mesh(np.asarray(devices), ("core",))
    n_in = len(in_names) + len(out_names)
    fn = jax.jit(
        shard_map(
            _body,
            mesh=mesh,
            in_specs=(PartitionSpec("core"),) * n_in,
            out_specs=(PartitionSpec("core"),) * len(out_names),
            check_rep=False,
        ),
        keep_unused=True,
    )
    _FN_CACHE = (fn, in_names, out_names, zero_outs)
    return _FN_CACHE


def kernel(**inputs: np.ndarray) -> np.ndarray:
    x = np.ascontiguousarray(inputs["x"], dtype=np.float32)
    w = np.ascontiguousarray(inputs["W"], dtype=np.float32)
    assert x.shape == (N_TOKENS, D_IN) and w.shape == (D_IN, D_OUT)

    fn, in_names, out_names, zero_outs = _get_callable()
    per_core = {"x": np.split(x, N_CORES, axis=0), "W": [w] * N_CORES}
    concat_in = [np.concatenate(per_core[name], axis=0) for name in in_names]
    concat_in += [np.concatenate([z] * N_CORES, axis=0) for z in zero_outs]
    outs = fn(*concat_in)
    out = np.asarray(outs[out_names.index("out")])
    return out.astype(np.float32)


if __name__ == "__main__":
    rng = np.random.default_rng(0)
    x = rng.standard_normal((N_TOKENS, D_IN), dtype=np.float32)
    w = rng.standard_normal((D_IN, D_OUT), dtype=np.float32)
    got = kernel(x=x, W=w)
    want = x @ np.sign(w)
    err = np.linalg.norm(got - want) / np.linalg.norm(want)
    print("rel err:", err)

